# revision 19
# baseline (speedup 1.0000x reference)
"""Self-attention (Q=K=V) Trainium2 Bass kernel.

Full input: inputs [8, 2048, 256] fp32.  Output: softmax(X X^T / 16) X,
batched over dim 0.  Sharding: pure data-parallel — one batch element
per NeuronCore (8 cores), no collectives.

Per-core algorithm (X = [2048, 256]):
  - Load X into SBUF row-block tiles (plus a ones column), build X^T
    on-chip via PE transposes; round both to fp32r so every matmul runs
    at full PE rate.
  - Stage 1 (per 512-wide column group g): for each 128-row block j,
    compute scores with fp32r matmuls and apply exp on the scalar
    engine, storing the unnormalized E = exp(S/16) row-blocks.
  - S is symmetric, so E's row-blocks double as the TRANSPOSED
    probability blocks stage 2 needs as stationary operands — the
    2048x2048 matrix is never transposed.
  - Stage 2 (per 128-query block i of group g): U_i = sum_j E_j[:, i]^T
    @ [X_j | 1].  The appended ones column accumulates the softmax
    denominator in the same PSUM tile, bit-consistent with the
    numerator weights.  Scale by its reciprocal and DMA out.
  - Stage-1 work of group g+1 is interleaved with stage-2 work of
    group g in PE emission order, so the scalar-engine exp stream
    overlaps the PE and the PE never idles (keeps the HAM clock warm).
"""

import numpy as np

import concourse.bacc as bacc
import concourse.tile as tile
from concourse import mybir
from concourse.bass_utils import run_bass_kernel_spmd
from concourse.masks import make_identity

B = 8
N = 2048
D = 256
P = 128
T = N // P   # 16 row/column tiles
C = D // P   # 2 contraction chunks for the scores matmul
G = 4        # 512-wide column groups
GW = N // G  # 512
IPG = T // G  # 4 output tiles per column group
SCALE = 1.0 / 16.0  # 1/sqrt(D)

F32 = mybir.dt.float32
F32R = mybir.dt.float32r


def _build_nc():
    nc = bacc.Bacc("TRN2", target_bir_lowering=False, debug=False, num_devices=B)
    x = nc.dram_tensor("x", [N, D], F32, kind="ExternalInput").ap()
    out = nc.dram_tensor("out", [N, D], F32, kind="ExternalOutput").ap()

    with tile.TileContext(nc) as tc:
        with (
            tc.tile_pool(name="big", bufs=1) as big,
            tc.tile_pool(name="small", bufs=1) as small,
            tc.tile_pool(name="psum", bufs=8, space="PSUM") as psum,
            tc.tile_pool(name="ot", bufs=4) as ot,
        ):
            # x_tiles[j][p, 0:256] = X[j*128+p, :]; col 256 = 1.0
            x_tiles = [
                big.tile([P, D + 2], F32, name=f"xj{j}", tag=f"x{j}")
                for j in range(T)
            ]
            xr_tiles = [
                big.tile([P, D + 2], F32R, name=f"xr{j}", tag=f"xr{j}")
                for j in range(T)
            ]
            xt_sb = big.tile([P, C, N], F32R)  # X^T: xt[p, c, n] = X[n, c*128+p]
            e_sb = big.tile([P, T, N], F32R)   # e_sb[p, j, i] = exp(S[j*128+p, i])

            # Input DMAs first (split across the two HWDGE queues).
            xv = x.rearrange("(t p) d -> p t d", p=P)
            for j in range(T):
                nc.vector.memset(x_tiles[j][:, D : D + 2], 1.0)
            for j in range(T):
                eng = nc.sync if j % 2 == 0 else nc.scalar
                eng.dma_start(out=x_tiles[j][:, 0:D], in_=xv[:, j, :])

            ident = small.tile([P, P], F32)
            make_identity(nc, ident)
            identr = small.tile([P, P], F32R)
            nc.vector.tensor_copy(identr[:], ident[:])

            def load_step(j):
                nc.vector.tensor_copy(xr_tiles[j][:], x_tiles[j][:])
                for c in range(C):
                    pt = psum.tile([P, P], F32R, tag="ps", name=f"pt{j}_{c}")
                    nc.tensor.transpose(
                        pt[:], xr_tiles[j][:, c * P : (c + 1) * P], identr[:]
                    )
                    nc.vector.tensor_copy(xt_sb[:, c, j * P : (j + 1) * P], pt[:])

            def t1_step(g, j):
                """Scores + exp for tile row j, column group g."""
                ps = psum.tile([P, GW], F32, tag="ps", name=f"ps{g}_{j}")
                for c in range(C):
                    nc.tensor.matmul(
                        ps[:],
                        lhsT=xt_sb[:, c, j * P : (j + 1) * P],
                        rhs=xt_sb[:, c, g * GW : (g + 1) * GW],
                        start=(c == 0),
                        stop=(c == C - 1),
                    )
                nc.scalar.activation(
                    out=e_sb[:, j, g * GW : (g + 1) * GW],
                    in_=ps[:],
                    func=mybir.ActivationFunctionType.Exp,
                    scale=SCALE,
                )

            out_r = out.rearrange("(t p) d -> p t d", p=P)
            s2_state = {}

            def s2_mm(g, i, j):
                """One stage-2 accumulation matmul for output tile i."""
                it = g * IPG + i
                if j == 0:
                    s2_state[it] = psum.tile(
                        [P, D + 2], F32, tag="ps", name=f"po{it}"
                    )
                po = s2_state[it]
                nc.tensor.matmul(
                    po[:],
                    lhsT=e_sb[:, j, it * P : (it + 1) * P],
                    rhs=xr_tiles[j][:],
                    start=(j == 0),
                    stop=(j == T - 1),
                )
                if j == T - 1:
                    rl = ot.tile([P, 1], F32, tag="rl", name=f"rl{it}")
                    nc.vector.reciprocal(rl[:], po[:, D : D + 1])
                    o_t = ot.tile([P, D], F32, tag="ot", name=f"o{it}")
                    nc.vector.tensor_scalar_mul(o_t[:], po[:, 0:D], rl[:])
                    nc.sync.dma_start(out=out_r[:, it, :], in_=o_t[:])

            # Software-pipelined emission: T1(g) runs interleaved with S2(g-1).
            # The X^T build is itself interleaved into T1(g0): t1(0, j) only
            # needs X^T blocks 0..3 (its rhs) plus block j (its lhsT).
            for j in range(4):
                load_step(j)
            for j in range(T):
                if j + 4 < T:
                    load_step(j + 4)
                t1_step(0, j)
            for g in range(1, G):
                # 64 S2 matmuls of group g-1 interleaved into 16 T1 steps of g
                s2_list = [(i, j) for i in range(IPG) for j in range(T)]
                for j in range(T):
                    t1_step(g, j)
                    for i2, j2 in s2_list[j * 4 : (j + 1) * 4]:
                        s2_mm(g - 1, i2, j2)
            for i in range(IPG):
                for j in range(T):
                    s2_mm(G - 1, i, j)

    nc.compile()
    return nc


_NC_CACHE = None


def kernel(inputs: np.ndarray) -> np.ndarray:
    global _NC_CACHE
    if _NC_CACHE is None:
        _NC_CACHE = _build_nc()
    nc = _NC_CACHE
    inputs = np.ascontiguousarray(np.asarray(inputs, dtype=np.float32))
    assert inputs.shape == (B, N, D)
    in_maps = [{"x": inputs[i]} for i in range(B)]
    res = run_bass_kernel_spmd(nc, in_maps, list(range(B)))
    return np.stack([res.results[i]["out"] for i in range(B)], axis=0)


# revision 20
# speedup vs baseline: 1.0369x; 1.0369x over previous
"""Self-attention (Q=K=V) Trainium2 Bass kernel.

Full input: inputs [8, 2048, 256] fp32.  Output: softmax(X X^T / 16) X,
batched over dim 0.  Sharding: pure data-parallel — one batch element
per NeuronCore (8 cores), no collectives.

Per-core algorithm (X = [2048, 256]):
  - Load X into SBUF row-block tiles (plus a ones column), build X^T
    on-chip via PE transposes; round both to fp32r so every matmul runs
    at full PE rate.
  - Stage 1 (per 512-wide column group g): for each 128-row block j,
    compute scores with fp32r matmuls and apply exp on the scalar
    engine, storing the unnormalized E = exp(S/16) row-blocks.
  - S is symmetric, so E's row-blocks double as the TRANSPOSED
    probability blocks stage 2 needs as stationary operands — the
    2048x2048 matrix is never transposed.
  - Stage 2 (per 128-query block i of group g): U_i = sum_j E_j[:, i]^T
    @ [X_j | 1].  The appended ones column accumulates the softmax
    denominator in the same PSUM tile, bit-consistent with the
    numerator weights.  Scale by its reciprocal and DMA out.
  - Stage-1 work of group g+1 is interleaved with stage-2 work of
    group g in PE emission order, so the scalar-engine exp stream
    overlaps the PE and the PE never idles (keeps the HAM clock warm).
"""

import numpy as np

import concourse.bacc as bacc
import concourse.tile as tile
from concourse import mybir
from concourse.bass_utils import run_bass_kernel_spmd
from concourse.masks import make_identity

B = 8
N = 2048
D = 256
P = 128
T = N // P   # 16 row/column tiles
C = D // P   # 2 contraction chunks for the scores matmul
G = 4        # 512-wide column groups
GW = N // G  # 512
IPG = T // G  # 4 output tiles per column group
SCALE = 1.0 / 16.0  # 1/sqrt(D)

F32 = mybir.dt.float32
F32R = mybir.dt.float32r


def _build_nc():
    nc = bacc.Bacc("TRN2", target_bir_lowering=False, debug=False, num_devices=B)
    x = nc.dram_tensor("x", [N, D], F32, kind="ExternalInput").ap()
    out = nc.dram_tensor("out", [N, D], F32, kind="ExternalOutput").ap()

    with tile.TileContext(nc) as tc:
        with (
            tc.tile_pool(name="big", bufs=1) as big,
            tc.tile_pool(name="small", bufs=1) as small,
            tc.tile_pool(name="psum", bufs=8, space="PSUM") as psum,
            tc.tile_pool(name="ot", bufs=4) as ot,
        ):
            # x_tiles[j][p, 0:256] = X[j*128+p, :]; col 256 = 1.0
            x_tiles = [
                big.tile([P, D + 2], F32, name=f"xj{j}", tag=f"x{j}")
                for j in range(T)
            ]
            xr_tiles = [
                big.tile([P, D + 2], F32R, name=f"xr{j}", tag=f"xr{j}")
                for j in range(T)
            ]
            xt_sb = big.tile([P, C, N], F32R)  # X^T: xt[p, c, n] = X[n, c*128+p]
            e_sb = big.tile([P, T, N], F32R)   # e_sb[p, j, i] = exp(S[j*128+p, i])

            # Input DMAs first (split across the two HWDGE queues).
            xv = x.rearrange("(t p) d -> p t d", p=P)
            for j in range(T):
                nc.vector.memset(x_tiles[j][:, D : D + 2], 1.0)
            for j in range(T):
                eng = nc.sync if j % 2 == 0 else nc.scalar
                eng.dma_start(out=x_tiles[j][:, 0:D], in_=xv[:, j, :])

            ident = small.tile([P, P], F32)
            make_identity(nc, ident)

            def load_step(j):
                nc.vector.tensor_copy(xr_tiles[j][:], x_tiles[j][:])
                for c in range(C):
                    pt = psum.tile([P, P], F32, tag="ps", name=f"pt{j}_{c}")
                    nc.tensor.transpose(
                        pt[:], x_tiles[j][:, c * P : (c + 1) * P], ident[:]
                    )
                    nc.vector.tensor_copy(xt_sb[:, c, j * P : (j + 1) * P], pt[:])

            def t1_step(g, j):
                """Scores + exp for tile row j, column group g."""
                ps = psum.tile([P, GW], F32, tag="ps", name=f"ps{g}_{j}")
                for c in range(C):
                    nc.tensor.matmul(
                        ps[:],
                        lhsT=xt_sb[:, c, j * P : (j + 1) * P],
                        rhs=xt_sb[:, c, g * GW : (g + 1) * GW],
                        start=(c == 0),
                        stop=(c == C - 1),
                    )
                nc.scalar.activation(
                    out=e_sb[:, j, g * GW : (g + 1) * GW],
                    in_=ps[:],
                    func=mybir.ActivationFunctionType.Exp,
                    scale=SCALE,
                )

            out_r = out.rearrange("(t p) d -> p t d", p=P)
            s2_state = {}

            def s2_mm(g, i, j):
                """One stage-2 accumulation matmul for output tile i."""
                it = g * IPG + i
                if j == 0:
                    s2_state[it] = psum.tile(
                        [P, D + 2], F32, tag="ps", name=f"po{it}"
                    )
                po = s2_state[it]
                nc.tensor.matmul(
                    po[:],
                    lhsT=e_sb[:, j, it * P : (it + 1) * P],
                    rhs=xr_tiles[j][:],
                    start=(j == 0),
                    stop=(j == T - 1),
                )
                if j == T - 1:
                    rl = ot.tile([P, 1], F32, tag="rl", name=f"rl{it}")
                    nc.vector.reciprocal(rl[:], po[:, D : D + 1])
                    o_t = ot.tile([P, D], F32, tag="ot", name=f"o{it}")
                    nc.vector.tensor_scalar_mul(o_t[:], po[:, 0:D], rl[:])
                    nc.sync.dma_start(out=out_r[:, it, :], in_=o_t[:])

            # Software-pipelined emission: T1(g) runs interleaved with S2(g-1).
            # The X^T build is itself interleaved into T1(g0): t1(0, j) only
            # needs X^T blocks 0..3 (its rhs) plus block j (its lhsT).
            for j in range(4):
                load_step(j)
            for j in range(T):
                if j + 4 < T:
                    load_step(j + 4)
                t1_step(0, j)
            for g in range(1, G):
                # 64 S2 matmuls of group g-1 interleaved into 16 T1 steps of g
                s2_list = [(i, j) for i in range(IPG) for j in range(T)]
                for j in range(T):
                    t1_step(g, j)
                    for i2, j2 in s2_list[j * 4 : (j + 1) * 4]:
                        s2_mm(g - 1, i2, j2)
            for i in range(IPG):
                for j in range(T):
                    s2_mm(G - 1, i, j)
            wp = psum.tile([P, P], F32, tag="ps", name="tailwarm")
            nc.tensor.matmul(
                wp[:], lhsT=ident[:], rhs=ident[:], start=True, stop=True
            )

    nc.compile()
    return nc


_NC_CACHE = None


def kernel(inputs: np.ndarray) -> np.ndarray:
    global _NC_CACHE
    if _NC_CACHE is None:
        _NC_CACHE = _build_nc()
    nc = _NC_CACHE
    inputs = np.ascontiguousarray(np.asarray(inputs, dtype=np.float32))
    assert inputs.shape == (B, N, D)
    in_maps = [{"x": inputs[i]} for i in range(B)]
    res = run_bass_kernel_spmd(nc, in_maps, list(range(B)))
    return np.stack([res.results[i]["out"] for i in range(B)], axis=0)
